# revision 29
# baseline (speedup 1.0000x reference)
"""Trainium2 Bass kernel for nn_AdaptiveSequenceProcessor.

Math (see reference):
  context  = mean_s(features)                               [B, D]
  span_w   = softmax(MLP_sp(context))                       [B, 3]
  feat_l   = relevance_pool(features[-32:],  context, 0.9)  [B, D]
  feat_g   = relevance_pool(features[::128], context, 0.8)  [B, D]
  fused    = LN(gelu(W_ff @ [feat_l*sw1 | feat_g*sw2]))     [B, D]
  gate     = sigmoid(W_g @ fused)  (constant over s!)       [B, D]
  out      = LN(features + fused*gate)                      [S, B, D]

Key structural fact: fused_expanded/gate are constant along the sequence
axis, so per batch element the heavy work is one mean over S (pass 1)
and one LN(features + c_b) sweep (pass 2); everything else is tiny.

Sharding: data-parallel over batch. 16 batch elements / 8 cores = 2 per
core, weights replicated, no collectives needed.
"""

import os
import sys

sys.path.insert(0, "/opt/trn_rl_repo")

import numpy as np
import ml_dtypes

import concourse.bass as bass
import concourse.tile as tile
from concourse import bacc, mybir
from concourse.bass_utils import run_bass_kernel_spmd

F32 = mybir.dt.float32
BF16 = mybir.dt.bfloat16
AF = mybir.ActivationFunctionType
ALU = mybir.AluOpType
AX = mybir.AxisListType

S, B, D, H = 4096, 16, 1024, 512
P = 128          # SBUF partitions
NC = 8           # cores
BPC = B // NC    # batch per core = 2
NT = S // P      # 32 sequence tiles of 128
L = 32           # pool history length
DC = D // P      # 8 d-chunks of 128
HC = H // P      # 4 h-chunks of 128
LN_EPS = 1e-5


def _build(apply_ln_affine: bool, use_ttr=False, use_sq_accum=False,
           use_pe_transpose=False):
    """Build the per-core Bass graph. Returns compiled Bacc."""
    nc = bacc.Bacc("TRN2", target_bir_lowering=False, debug=False,
                   num_devices=NC)

    feat = nc.declare_dram_parameter("features", [S, BPC, D], F32, isOutput=False)
    w_sp1t = nc.declare_dram_parameter("w_sp1t", [D, H], BF16, isOutput=False)
    b_sp1t = nc.declare_dram_parameter("b_sp1t", [P, HC], F32, isOutput=False)
    w_sp2t = nc.declare_dram_parameter("w_sp2t", [H, 3], BF16, isOutput=False)
    b_sp2 = nc.declare_dram_parameter("b_sp2", [1, 3], F32, isOutput=False)
    w_rs1t = nc.declare_dram_parameter("w_rs1t", [2 * D, H], BF16, isOutput=False)
    b_rs1 = nc.declare_dram_parameter("b_rs1", [1, H], F32, isOutput=False)
    w_rs2r = nc.declare_dram_parameter("w_rs2r", [1, H], F32, isOutput=False)
    dlb = nc.declare_dram_parameter("dlb", [1, L], F32, isOutput=False)
    dgb = nc.declare_dram_parameter("dgb", [1, L], F32, isOutput=False)
    w_fft = nc.declare_dram_parameter("w_fft", [2 * D, D], BF16, isOutput=False)
    b_ff = nc.declare_dram_parameter("b_ff", [1, D], F32, isOutput=False)
    lnffg = nc.declare_dram_parameter("lnffg", [1, D], F32, isOutput=False)
    lnffb = nc.declare_dram_parameter("lnffb", [1, D], F32, isOutput=False)
    w_gatet = nc.declare_dram_parameter("w_gatet", [D, D], BF16, isOutput=False)
    b_gate = nc.declare_dram_parameter("b_gate", [1, D], F32, isOutput=False)
    i32d = nc.declare_dram_parameter("i32", [L, L], F32, isOutput=False)
    if apply_ln_affine:
        lng = nc.declare_dram_parameter("lng", [1, D], F32, isOutput=False)
        lnb = nc.declare_dram_parameter("lnb", [1, D], F32, isOutput=False)
    out = nc.declare_dram_parameter("out", [S, BPC, D], F32, isOutput=True)

    with tile.TileContext(nc) as tc:
        from contextlib import ExitStack
        with ExitStack() as ctx:
            consts = ctx.enter_context(tc.tile_pool(name="consts", bufs=1))
            wpool = ctx.enter_context(tc.tile_pool(name="weights", bufs=1))
            bpool = ctx.enter_context(tc.tile_pool(name="perb", bufs=1))
            scp = ctx.enter_context(tc.tile_pool(name="sc", bufs=1))
            rows = ctx.enter_context(tc.tile_pool(name="rows", bufs=2))
            p1p = ctx.enter_context(tc.tile_pool(name="p1", bufs=4))
            p2p = ctx.enter_context(tc.tile_pool(name="p2", bufs=3))
            stp = ctx.enter_context(tc.tile_pool(name="stats", bufs=3))
            psS = ctx.enter_context(tc.tile_pool(name="psS", bufs=5, space="PSUM"))
            psC = ctx.enter_context(tc.tile_pool(name="psC", bufs=1, space="PSUM"))

            # ---- constants ----
            ones128 = consts.tile([P, 1], F32)
            nc.vector.memset(ones128[:], 1.0)
            one11 = consts.tile([1, 1], F32)
            nc.vector.memset(one11[:], 1.0)
            ones_1x32_bf = consts.tile([1, L], BF16)
            nc.vector.memset(ones_1x32_bf[:], 1.0)
            ones_1x128 = consts.tile([1, P], F32)
            nc.vector.memset(ones_1x128[:], 1.0)
            i32sb = consts.tile([L, L], F32)
            nc.sync.dma_start(i32sb[:], i32d[:])
            eps_sb = consts.tile([P, 1], F32)
            nc.vector.memset(eps_sb[:], LN_EPS)

            # ---- weights to SBUF ----
            w_sp1t_sb = wpool.tile([P, DC, H], BF16)
            nc.sync.dma_start(w_sp1t_sb[:], w_sp1t.ap().rearrange("(c p) n -> p c n", p=P))
            w_sp2t_sb = wpool.tile([P, HC, 3], BF16)
            nc.sync.dma_start(w_sp2t_sb[:], w_sp2t.ap().rearrange("(c p) n -> p c n", p=P))
            w_rs1t_sb = wpool.tile([P, 2 * DC, H], BF16)
            nc.sync.dma_start(w_rs1t_sb[:], w_rs1t.ap().rearrange("(c p) n -> p c n", p=P))
            w_fft_sb = wpool.tile([P, 2 * DC, D], BF16)
            nc.sync.dma_start(w_fft_sb[:], w_fft.ap().rearrange("(c p) n -> p c n", p=P))
            w_gatet_sb = wpool.tile([P, DC, D], BF16)
            nc.sync.dma_start(w_gatet_sb[:], w_gatet.ap().rearrange("(c p) n -> p c n", p=P))
            b_sp1t_sb = wpool.tile([P, HC], F32)
            nc.sync.dma_start(b_sp1t_sb[:], b_sp1t[:])

            b_sp2_sb = consts.tile([1, 3], F32)
            nc.sync.dma_start(b_sp2_sb[:], b_sp2[:])
            b_rs1_sb = consts.tile([1, H], F32)
            nc.sync.dma_start(b_rs1_sb[:], b_rs1[:])
            w_rs2r_sb = consts.tile([1, H], F32)
            nc.sync.dma_start(w_rs2r_sb[:], w_rs2r[:])
            dlb_sb = consts.tile([1, L], F32)
            nc.sync.dma_start(dlb_sb[:], dlb[:])
            dgb_sb = consts.tile([1, L], F32)
            nc.sync.dma_start(dgb_sb[:], dgb[:])
            b_ff_sb = consts.tile([1, D], F32)
            nc.sync.dma_start(b_ff_sb[:], b_ff[:])
            lnffg_sb = consts.tile([1, D], F32)
            nc.sync.dma_start(lnffg_sb[:], lnffg[:])
            lnffb_sb = consts.tile([1, D], F32)
            nc.sync.dma_start(lnffb_sb[:], lnffb[:])
            b_gate_sb = consts.tile([1, D], F32)
            nc.sync.dma_start(b_gate_sb[:], b_gate[:])

            # w_rs2 broadcast to 32 partitions (one-time)
            pw32 = psS.tile([L, H], F32, tag="ps")
            ones_1x32_f = consts.tile([1, L], F32)
            nc.vector.memset(ones_1x32_f[:], 1.0)
            nc.tensor.matmul(pw32[:], lhsT=ones_1x32_f[:], rhs=w_rs2r_sb[:],
                             start=True, stop=True)
            w_rs2b = consts.tile([L, H], F32)
            nc.scalar.copy(w_rs2b[:], pw32[:])

            if apply_ln_affine:
                lng_sb = consts.tile([1, D], F32)
                nc.sync.dma_start(lng_sb[:], lng[:])
                lnb_sb = consts.tile([1, D], F32)
                nc.sync.dma_start(lnb_sb[:], lnb[:])
                g_bcast = bpool.tile([P, D], F32)
                b_bcast = bpool.tile([P, D], F32)
                for h2 in range(2):
                    pgb = psS.tile([P, 512], F32, tag="ps")
                    nc.tensor.matmul(pgb[:], lhsT=ones_1x128[:],
                                     rhs=lng_sb[:, h2 * 512:(h2 + 1) * 512],
                                     start=True, stop=True)
                    nc.scalar.copy(g_bcast[:, h2 * 512:(h2 + 1) * 512], pgb[:])
                    pbb = psS.tile([P, 512], F32, tag="ps")
                    nc.tensor.matmul(pbb[:], lhsT=ones_1x128[:],
                                     rhs=lnb_sb[:, h2 * 512:(h2 + 1) * 512],
                                     start=True, stop=True)
                    nc.scalar.copy(b_bcast[:, h2 * 512:(h2 + 1) * 512], pbb[:])

            c_bcast = [bpool.tile([P, D], F32, tag=f"cb{b}", name=f"c_bcast{b}")
                       for b in range(BPC)]
            ctxT_bf = [bpool.tile([P, DC], BF16, tag=f"ctxT{b}", name=f"ctxT{b}")
                       for b in range(BPC)]

            def pe_transpose(out_psum, in_sb, ident):
                if use_pe_transpose:
                    nc.tensor.transpose(out_psum, in_sb, ident)
                else:
                    nc.tensor.matmul(out_psum, lhsT=in_sb, rhs=ident,
                                     start=True, stop=True)

            # ================= PASS 1: context mean =================
            for b in range(BPC):
                pc = psC.tile([1, D], F32, tag="pc")  # 2 banks
                for i in range(NT):
                    x = p1p.tile([P, D], F32, tag="x")
                    nc.sync.dma_start(x[:], feat[i * P:(i + 1) * P, b, :])
                    for h2 in range(2):
                        nc.tensor.matmul(pc[:, h2 * 512:(h2 + 1) * 512],
                                         lhsT=ones128[:],
                                         rhs=x[:, h2 * 512:(h2 + 1) * 512],
                                         start=(i == 0), stop=(i == NT - 1))
                ctx_row = scp.tile([1, D], F32, tag="ctxrow")
                nc.scalar.mul(ctx_row[:], pc[:], 1.0 / S)
                pct = psS.tile([P, DC], F32, tag="ps")
                for j in range(DC):
                    pe_transpose(pct[:, j:j + 1],
                                 ctx_row[:, j * P:(j + 1) * P], one11[:])
                nc.scalar.copy(ctxT_bf[b][:], pct[:])

            # ================= SMALL COMPUTE per b =================
            def small_compute(b):
                # --- span MLP ---
                psp = psS.tile([P, HC], F32, tag="ps")
                for jh in range(HC):
                    for jd in range(DC):
                        nc.tensor.matmul(
                            psp[:, jh:jh + 1],
                            lhsT=w_sp1t_sb[:, jd, jh * P:(jh + 1) * P],
                            rhs=ctxT_bf[b][:, jd:jd + 1],
                            start=(jd == 0), stop=(jd == DC - 1))
                sp_hT = scp.tile([P, HC], BF16, tag="sphT")
                for jh in range(HC):
                    nc.scalar.activation(sp_hT[:, jh:jh + 1], psp[:, jh:jh + 1],
                                         AF.Gelu, bias=b_sp1t_sb[:, jh:jh + 1])
                psl = psS.tile([1, 3], F32, tag="ps")
                for jh in range(HC):
                    nc.tensor.matmul(psl[:], lhsT=sp_hT[:, jh:jh + 1],
                                     rhs=w_sp2t_sb[:, jh, :],
                                     start=(jh == 0), stop=(jh == HC - 1))
                sl = rows.tile([1, 3], F32, tag="sl")
                nc.vector.tensor_add(sl[:], psl[:], b_sp2_sb[:])
                mx = stp.tile([1, 1], F32, tag="mx")
                nc.vector.reduce_max(mx[:], sl[:], AX.X)
                e3 = rows.tile([1, 3], F32, tag="e3")
                nc.vector.tensor_scalar(e3[:], sl[:], mx[:], None, op0=ALU.subtract)
                nc.scalar.activation(e3[:], e3[:], AF.Exp)
                z3 = stp.tile([1, 1], F32, tag="z3")
                nc.vector.reduce_sum(z3[:], e3[:], AX.X)
                rz3 = stp.tile([1, 1], F32, tag="rz3")
                nc.vector.reciprocal(rz3[:], z3[:])
                sw = rows.tile([1, 3], F32, tag="sw")
                nc.vector.tensor_scalar(sw[:], e3[:], rz3[:], None, op0=ALU.mult)

                # --- ctx part of relevance-MLP (shared by both pools) ---
                pcr = psS.tile([1, H], F32, tag="ps")
                for jd in range(DC):
                    nc.tensor.matmul(pcr[:], lhsT=ctxT_bf[b][:, jd:jd + 1],
                                     rhs=w_rs1t_sb[:, DC + jd, :],
                                     start=(jd == 0), stop=(jd == DC - 1))
                ctx_rs_bf = scp.tile([1, H], BF16, tag="ctxrs")
                nc.vector.tensor_add(ctx_rs_bf[:], pcr[:], b_rs1_sb[:])

                # --- two relevance pools ---
                featT = []
                for pi, dbias in enumerate([dlb_sb, dgb_sb]):
                    hist = scp.tile([L, D], F32, tag="hist")
                    if pi == 0:
                        nc.sync.dma_start(hist[:], feat[S - L:S, b, :])
                    else:
                        nc.sync.dma_start(hist[:], feat[0:S:P, b, :])
                    hist_bf = scp.tile([L, D], BF16, tag="histbf")
                    nc.scalar.copy(hist_bf[:], hist[:])
                    histT_bf = scp.tile([P, DC, L], BF16, tag="histT")
                    for jd in range(DC):
                        pt = psS.tile([P, L], F32, tag="ps")
                        pe_transpose(pt[:], hist[:, jd * P:(jd + 1) * P],
                                     i32sb[:])
                        nc.scalar.copy(histT_bf[:, jd, :], pt[:])
                    ph = psS.tile([L, H], F32, tag="ps")
                    for jd in range(DC):
                        nc.tensor.matmul(ph[:], lhsT=histT_bf[:, jd, :],
                                         rhs=w_rs1t_sb[:, jd, :],
                                         start=(jd == 0), stop=False)
                    nc.tensor.matmul(ph[:], lhsT=ones_1x32_bf[:],
                                     rhs=ctx_rs_bf[:], start=False, stop=True)
                    h_sb = scp.tile([L, H], F32, tag="h")
                    nc.scalar.activation(h_sb[:], ph[:], AF.Gelu)
                    hw = scp.tile([L, H], F32, tag="hw")
                    sc_col = stp.tile([L, 1], F32, tag="sccol")
                    if use_ttr:
                        nc.vector.tensor_tensor_reduce(
                            out=hw[:], in0=h_sb[:], in1=w_rs2b[:], scale=1.0,
                            scalar=0.0, op0=ALU.mult, op1=ALU.add,
                            accum_out=sc_col[:])
                    else:
                        nc.vector.tensor_mul(hw[:], h_sb[:], w_rs2b[:])
                        nc.vector.reduce_sum(sc_col[:], hw[:], AX.X)
                    pst = psS.tile([1, L], F32, tag="ps")
                    pe_transpose(pst[:], sc_col[:], i32sb[:])
                    srow = rows.tile([1, L], F32, tag="srow")
                    nc.vector.tensor_add(srow[:], pst[:], dbias[:])
                    mx2 = stp.tile([1, 1], F32, tag="mx2")
                    nc.vector.reduce_max(mx2[:], srow[:], AX.X)
                    e32 = rows.tile([1, L], F32, tag="e32")
                    nc.vector.tensor_scalar(e32[:], srow[:], mx2[:], None,
                                            op0=ALU.subtract)
                    nc.scalar.activation(e32[:], e32[:], AF.Exp)
                    z32 = stp.tile([1, 1], F32, tag="z32")
                    nc.vector.reduce_sum(z32[:], e32[:], AX.X)
                    rz32 = stp.tile([1, 1], F32, tag="rz32")
                    nc.vector.reciprocal(rz32[:], z32[:])
                    wrow = rows.tile([1, L], F32, tag="wrow")
                    nc.vector.tensor_scalar(wrow[:], e32[:], rz32[:], None,
                                            op0=ALU.mult)
                    pwc = psS.tile([L, 1], F32, tag="ps")
                    pe_transpose(pwc[:], wrow[:], one11[:])
                    wcol_bf = stp.tile([L, 1], BF16, tag="wcol")
                    nc.scalar.copy(wcol_bf[:], pwc[:])
                    pft = psS.tile([P, DC], F32, tag="ps")
                    for jd in range(DC):
                        nc.tensor.matmul(pft[:, jd:jd + 1],
                                         lhsT=hist_bf[:, jd * P:(jd + 1) * P],
                                         rhs=wcol_bf[:], start=True, stop=True)
                    fT = scp.tile([P, DC], BF16, tag=f"featT{pi}")
                    nc.scalar.copy(fT[:], pft[:])
                    featT.append(fT)

                # --- fusion ff ---
                pAB = []
                for pi in range(2):
                    pAh = []
                    for h2 in range(2):
                        pA = psS.tile([1, 512], F32, tag="ps")
                        for jd in range(DC):
                            nc.tensor.matmul(
                                pA[:], lhsT=featT[pi][:, jd:jd + 1],
                                rhs=w_fft_sb[:, pi * DC + jd,
                                             h2 * 512:(h2 + 1) * 512],
                                start=(jd == 0), stop=(jd == DC - 1))
                        pAh.append(pA)
                    pAB.append(pAh)
                t1 = scp.tile([1, D], F32, tag="rowA")
                t2 = scp.tile([1, D], F32, tag="rowB")
                for h2 in range(2):
                    nc.vector.tensor_scalar(t1[:, h2 * 512:(h2 + 1) * 512],
                                            pAB[0][h2][:], sw[:, 1:2], None,
                                            op0=ALU.mult)
                    nc.vector.tensor_scalar(t2[:, h2 * 512:(h2 + 1) * 512],
                                            pAB[1][h2][:], sw[:, 2:3], None,
                                            op0=ALU.mult)
                nc.vector.tensor_add(t1[:], t1[:], t2[:])
                nc.vector.tensor_add(t1[:], t1[:], b_ff_sb[:])
                ff = scp.tile([1, D], F32, tag="rowC")
                nc.scalar.activation(ff[:], t1[:], AF.Gelu)
                # LN over free axis
                smu = stp.tile([1, 1], F32, tag="smu")
                nc.vector.reduce_sum(smu[:], ff[:], AX.X)
                mu = stp.tile([1, 1], F32, tag="mu")
                nc.vector.tensor_scalar(mu[:], smu[:], 1.0 / D, None, op0=ALU.mult)
                ffc = scp.tile([1, D], F32, tag="rowB")
                nc.vector.tensor_scalar(ffc[:], ff[:], mu[:], None, op0=ALU.subtract)
                ffsq = scp.tile([1, D], F32, tag="rowA")
                vs = stp.tile([1, 1], F32, tag="vs")
                if use_ttr:
                    nc.vector.tensor_tensor_reduce(
                        out=ffsq[:], in0=ffc[:], in1=ffc[:], scale=1.0,
                        scalar=0.0, op0=ALU.mult, op1=ALU.add, accum_out=vs[:])
                else:
                    nc.vector.tensor_mul(ffsq[:], ffc[:], ffc[:])
                    nc.vector.reduce_sum(vs[:], ffsq[:], AX.X)
                stdv = stp.tile([1, 1], F32, tag="stdv")
                nc.scalar.activation(stdv[:], vs[:], AF.Sqrt, bias=eps_sb[0:1, :],
                                     scale=1.0 / D)
                rstd = stp.tile([1, 1], F32, tag="rstd")
                nc.vector.reciprocal(rstd[:], stdv[:])
                fused = scp.tile([1, D], F32, tag="rowD")
                nc.vector.tensor_scalar(fused[:], ffc[:], rstd[:], None, op0=ALU.mult)
                nc.vector.tensor_mul(fused[:], fused[:], lnffg_sb[:])
                nc.vector.tensor_add(fused[:], fused[:], lnffb_sb[:])

                # --- gate ---
                pfT = psS.tile([P, DC], F32, tag="ps")
                for jd in range(DC):
                    pe_transpose(pfT[:, jd:jd + 1],
                                 fused[:, jd * P:(jd + 1) * P], one11[:])
                fusedT_bf = scp.tile([P, DC], BF16, tag="fusedT")
                nc.scalar.copy(fusedT_bf[:], pfT[:])
                gpre = scp.tile([1, D], F32, tag="rowA")
                for h2 in range(2):
                    pg = psS.tile([1, 512], F32, tag="ps")
                    for jd in range(DC):
                        nc.tensor.matmul(pg[:],
                                         lhsT=fusedT_bf[:, jd:jd + 1],
                                         rhs=w_gatet_sb[:, jd, h2 * 512:(h2 + 1) * 512],
                                         start=(jd == 0), stop=(jd == DC - 1))
                    nc.vector.tensor_add(gpre[:, h2 * 512:(h2 + 1) * 512], pg[:],
                                         b_gate_sb[:, h2 * 512:(h2 + 1) * 512])
                gate = scp.tile([1, D], F32, tag="rowB")
                nc.scalar.activation(gate[:], gpre[:], AF.Sigmoid)
                c_row = scp.tile([1, D], F32, tag="rowA")
                nc.vector.tensor_mul(c_row[:], fused[:], gate[:])
                for h2 in range(2):
                    pcb = psS.tile([P, 512], F32, tag="ps")
                    nc.tensor.matmul(pcb[:], lhsT=ones_1x128[:],
                                     rhs=c_row[:, h2 * 512:(h2 + 1) * 512],
                                     start=True, stop=True)
                    nc.scalar.copy(c_bcast[b][:, h2 * 512:(h2 + 1) * 512], pcb[:])

            # ================= PASS 2: LN(x + c) =================
            def pass2(b):
                for i in range(NT):
                    x = p2p.tile([P, D], F32, tag="x2")
                    nc.sync.dma_start(x[:], feat[i * P:(i + 1) * P, b, :])
                    xa = p2p.tile([P, D], F32, tag="xa")
                    o = p2p.tile([P, D], F32, tag="o")
                    if use_ttr and use_sq_accum:
                        st = stp.tile([P, 8], F32, tag="st")
                        nc.vector.tensor_tensor_reduce(
                            out=xa[:], in0=x[:], in1=c_bcast[b][:], scale=1.0,
                            scalar=0.0, op0=ALU.add, op1=ALU.add,
                            accum_out=st[:, 0:1])
                        nc.scalar.activation(o[:], xa[:], AF.Square,
                                             accum_out=st[:, 1:2])
                        # mean, E[x^2], var, rstd, -mu*rstd
                        nc.vector.tensor_scalar(st[:, 2:3], st[:, 0:1], 1.0 / D,
                                                None, op0=ALU.mult)
                        nc.vector.tensor_scalar(st[:, 3:4], st[:, 1:2], 1.0 / D,
                                                None, op0=ALU.mult)
                        nc.vector.tensor_mul(st[:, 4:5], st[:, 2:3], st[:, 2:3])
                        nc.vector.tensor_sub(st[:, 5:6], st[:, 3:4], st[:, 4:5])
                        nc.scalar.activation(st[:, 6:7], st[:, 5:6], AF.Sqrt,
                                             bias=eps_sb[:])
                        rst = stp.tile([P, 2], F32, tag="rst")
                        nc.vector.reciprocal(rst[:, 0:1], st[:, 6:7])
                        nc.vector.tensor_mul(rst[:, 1:2], st[:, 2:3], rst[:, 0:1])
                        nc.vector.tensor_scalar(rst[:, 1:2], rst[:, 1:2], -1.0,
                                                None, op0=ALU.mult)
                        nc.scalar.activation(o[:], xa[:], AF.Identity,
                                             bias=rst[:, 1:2], scale=rst[:, 0:1])
                    else:
                        nc.vector.tensor_add(xa[:], x[:], c_bcast[b][:])
                        stats = stp.tile([P, 2, 6], F32, tag="bst")
                        xa3 = xa.rearrange("p (s f) -> p s f", f=512)
                        for si in range(2):
                            nc.vector.bn_stats(stats[:, si, :], xa3[:, si, :])
                        mv = stp.tile([P, 2], F32, tag="mv")
                        nc.vector.bn_aggr(mv[:], stats[:])
                        rst = stp.tile([P, 1], F32, tag="rst2")
                        nc.scalar.activation(rst[:], mv[:, 1:2], AF.Sqrt,
                                             bias=eps_sb[:])
                        nc.vector.reciprocal(rst[:], rst[:])
                        nc.vector.tensor_scalar(o[:], xa[:], mv[:, 0:1],
                                                rst[:], op0=ALU.subtract,
                                                op1=ALU.mult)
                    if apply_ln_affine:
                        nc.vector.tensor_mul(o[:], o[:], g_bcast[:])
                        nc.vector.tensor_add(o[:], o[:], b_bcast[:])
                    nc.sync.dma_start(out[i * P:(i + 1) * P, b, :], o[:])

            for b in range(BPC):
                small_compute(b)
            for b in range(BPC):
                pass2(b)

            if os.environ.get("KERNEL_DEBUG_SIZES"):
                tot = 0
                for pl in [consts, wpool, bpool, scp, rows, p1p, p2p, stp]:
                    sz = pl.current_size() / 128 / 1024
                    tot += sz
                    print(f"pool {pl.name}: {sz:.1f} KB/partition")
                print(f"TOTAL SBUF: {tot:.1f} KB/partition of "
                      f"{nc.SBUF_PARTITION_SIZE_BYTES/1024:.0f}")
                for pl in [psS, psC]:
                    print(f"pool {pl.name}: {pl.current_size()/128/2048:.1f} banks")

    nc.compile()
    return nc


_CACHE = {}


def _get_nc(apply_ln_affine: bool):
    key = apply_ln_affine
    if key not in _CACHE:
        _CACHE[key] = _build(apply_ln_affine)
    return _CACHE[key]


def build_in_maps(inputs):
    features = np.asarray(inputs["features"], np.float32)
    f32 = lambda a: np.ascontiguousarray(np.asarray(a, np.float32))
    bf = lambda a: np.ascontiguousarray(
        np.asarray(a, np.float32).astype(ml_dtypes.bfloat16))

    b_rs2 = f32(inputs["b_rs2"])
    ln_g, ln_b = f32(inputs["ln_g"]), f32(inputs["ln_b"])
    dl = float(np.asarray(inputs["decay_local"]))
    dg = float(np.asarray(inputs["decay_global"]))
    apply_ln_affine = not (np.all(ln_g == 1.0) and np.all(ln_b == 0.0))

    pos = np.arange(L, dtype=np.float64)
    dlb = (np.log(dl ** (L - 1 - pos) + 1e-8) + float(b_rs2[0])).astype(np.float32)
    dgb = (np.log(dg ** (L - 1 - pos) + 1e-8) + float(b_rs2[0])).astype(np.float32)

    common = {
        "w_sp1t": bf(f32(inputs["w_sp1"]).T),
        "b_sp1t": f32(f32(inputs["b_sp1"]).reshape(HC, P).T),
        "w_sp2t": bf(f32(inputs["w_sp2"]).T),
        "b_sp2": f32(inputs["b_sp2"]).reshape(1, 3),
        "w_rs1t": bf(f32(inputs["w_rs1"]).T),
        "b_rs1": f32(inputs["b_rs1"]).reshape(1, H),
        "w_rs2r": f32(inputs["w_rs2"]).reshape(1, H),
        "dlb": dlb.reshape(1, L),
        "dgb": dgb.reshape(1, L),
        "w_fft": bf(f32(inputs["w_ff"]).T),
        "b_ff": f32(inputs["b_ff"]).reshape(1, D),
        "lnffg": f32(inputs["ln_ff_g"]).reshape(1, D),
        "lnffb": f32(inputs["ln_ff_b"]).reshape(1, D),
        "w_gatet": bf(f32(inputs["w_gate"]).T),
        "b_gate": f32(inputs["b_gate"]).reshape(1, D),
        "i32": np.eye(L, dtype=np.float32),
    }
    if apply_ln_affine:
        common["lng"] = ln_g.reshape(1, D)
        common["lnb"] = ln_b.reshape(1, D)

    in_maps = []
    for c in range(NC):
        m = dict(common)
        m["features"] = np.ascontiguousarray(features[:, c * BPC:(c + 1) * BPC, :])
        in_maps.append(m)
    return in_maps


def kernel(features, w_sp1, b_sp1, w_sp2, b_sp2, w_rs1, b_rs1, w_rs2, b_rs2,
           decay_local, decay_global, w_ff, b_ff, ln_ff_g, ln_ff_b,
           w_gate, b_gate, ln_g, ln_b, _trace=False):
    inputs = dict(features=features, w_sp1=w_sp1, b_sp1=b_sp1, w_sp2=w_sp2,
                  b_sp2=b_sp2, w_rs1=w_rs1, b_rs1=b_rs1, w_rs2=w_rs2,
                  b_rs2=b_rs2, decay_local=decay_local,
                  decay_global=decay_global, w_ff=w_ff, b_ff=b_ff,
                  ln_ff_g=ln_ff_g, ln_ff_b=ln_ff_b, w_gate=w_gate,
                  b_gate=b_gate, ln_g=ln_g, ln_b=ln_b)
    features = np.asarray(features, np.float32)
    ln_g_np = np.asarray(ln_g, np.float32)
    ln_b_np = np.asarray(ln_b, np.float32)
    apply_ln_affine = not (np.all(ln_g_np == 1.0) and np.all(ln_b_np == 0.0))
    nc = _get_nc(apply_ln_affine)
    in_maps = build_in_maps(inputs)

    res = run_bass_kernel_spmd(nc, in_maps, core_ids=list(range(NC)),
                               trace=_trace)
    output = np.concatenate([res.results[c]["out"] for c in range(NC)], axis=1)
    attention_weights = np.full((S, B), 1.0 / S, dtype=features.dtype)
    if _trace:
        kernel.last_exec_time_ns = res.exec_time_ns
        kernel.last_trace = res.instructions_and_trace
    return output, attention_weights


# revision 34
# speedup vs baseline: 159.0011x; 159.0011x over previous
"""Trainium2 Bass kernel for nn_AdaptiveSequenceProcessor.

Math (see reference):
  context  = mean_s(features)                               [B, D]
  span_w   = softmax(MLP_sp(context))                       [B, 3]
  feat_l   = relevance_pool(features[-32:],  context, 0.9)  [B, D]
  feat_g   = relevance_pool(features[::128], context, 0.8)  [B, D]
  fused    = LN(gelu(W_ff @ [feat_l*sw1 | feat_g*sw2]))     [B, D]
  gate     = sigmoid(W_g @ fused)  (constant over s!)       [B, D]
  out      = LN(features + fused*gate)                      [S, B, D]

Key structural fact: fused_expanded/gate are constant along the sequence
axis, so per batch element the heavy work is one mean over S (pass 1)
and one LN(features + c_b) sweep (pass 2); everything else is tiny.

Sharding: data-parallel over batch. 16 batch elements / 8 cores = 2 per
core, weights replicated, no collectives needed.

With FEAT_BF16: features are pre-cast to bf16 on host (halves the HBM
read), pass-1 tiles stay resident in SBUF and pass 2 reads them from
SBUF (no second HBM read). w_fft is streamed from DRAM per use to make
room for the cache.
"""

import os
import sys

sys.path.insert(0, "/opt/trn_rl_repo")

import numpy as np
import ml_dtypes

import concourse.bass as bass
import concourse.tile as tile
from concourse import bacc, mybir
from concourse.bass_utils import run_bass_kernel_spmd

F32 = mybir.dt.float32
BF16 = mybir.dt.bfloat16
AF = mybir.ActivationFunctionType
ALU = mybir.AluOpType
AX = mybir.AxisListType

S, B, D, H = 4096, 16, 1024, 512
P = 128          # SBUF partitions
NC = 8           # cores
BPC = B // NC    # batch per core = 2
NT = S // P      # 32 sequence tiles of 128
L = 32           # pool history length
DC = D // P      # 8 d-chunks of 128
HC = H // P      # 4 h-chunks of 128
LN_EPS = 1e-5
CACHE_SLOTS = 36

# Feature flags (module-level so test.py can flip them for experiments)
FLAGS = dict(
    feat_bf16=True,    # bf16 features input + SBUF cache + streamed w_fft
    use_ttr=False,     # tensor_tensor_reduce fused ops
    use_sq_accum=False,  # ACT Square with accum_out + Identity bias/scale apply
    use_pe_transpose=False,  # is_transpose matmuls instead of identity matmul
)


def _build(apply_ln_affine: bool, feat_bf16: bool, use_ttr: bool,
           use_sq_accum: bool, use_pe_transpose: bool, reps: int = 1):
    """Build the per-core Bass graph. Returns compiled Bacc.

    reps > 1 wraps the whole kernel in a tc.For_i hardware loop — used
    only by the timing harness to amortize per-call dispatch overhead.
    """
    nc = bacc.Bacc("TRN2", target_bir_lowering=False, debug=False,
                   num_devices=NC)

    FDT = BF16 if feat_bf16 else F32

    feat = nc.declare_dram_parameter("features", [S, BPC, D], FDT, isOutput=False)
    w_sp1t = nc.declare_dram_parameter("w_sp1t", [D, H], BF16, isOutput=False)
    b_sp1t = nc.declare_dram_parameter("b_sp1t", [P, HC], F32, isOutput=False)
    w_sp2t = nc.declare_dram_parameter("w_sp2t", [H, 3], BF16, isOutput=False)
    b_sp2 = nc.declare_dram_parameter("b_sp2", [1, 3], F32, isOutput=False)
    w_rs1t = nc.declare_dram_parameter("w_rs1t", [2 * D, H], BF16, isOutput=False)
    b_rs1 = nc.declare_dram_parameter("b_rs1", [1, H], F32, isOutput=False)
    w_rs2r = nc.declare_dram_parameter("w_rs2r", [1, H], F32, isOutput=False)
    dlb = nc.declare_dram_parameter("dlb", [1, L], F32, isOutput=False)
    dgb = nc.declare_dram_parameter("dgb", [1, L], F32, isOutput=False)
    w_fft = nc.declare_dram_parameter("w_fft", [2 * D, D], BF16, isOutput=False)
    b_ff = nc.declare_dram_parameter("b_ff", [1, D], F32, isOutput=False)
    lnffg = nc.declare_dram_parameter("lnffg", [1, D], F32, isOutput=False)
    lnffb = nc.declare_dram_parameter("lnffb", [1, D], F32, isOutput=False)
    w_gatet = nc.declare_dram_parameter("w_gatet", [D, D], BF16, isOutput=False)
    b_gate = nc.declare_dram_parameter("b_gate", [1, D], F32, isOutput=False)
    i32d = nc.declare_dram_parameter("i32", [L, L], F32, isOutput=False)
    if feat_bf16:
        i32bd = nc.declare_dram_parameter("i32b", [L, L], BF16, isOutput=False)
    if apply_ln_affine:
        lng = nc.declare_dram_parameter("lng", [1, D], F32, isOutput=False)
        lnb = nc.declare_dram_parameter("lnb", [1, D], F32, isOutput=False)
    out = nc.declare_dram_parameter("out", [S, BPC, D], F32, isOutput=True)

    w_fft_re = w_fft.ap().rearrange("(c p) n -> p c n", p=P)

    with tile.TileContext(nc) as tc:
        from contextlib import ExitStack
        with ExitStack() as ctx:
            consts = ctx.enter_context(tc.tile_pool(name="consts", bufs=1))
            wpool = ctx.enter_context(tc.tile_pool(name="weights", bufs=1))
            bpool = ctx.enter_context(tc.tile_pool(name="perb", bufs=1))
            scp = ctx.enter_context(tc.tile_pool(name="sc", bufs=1))
            rows = ctx.enter_context(tc.tile_pool(name="rows", bufs=2))
            if feat_bf16:
                cachep = ctx.enter_context(tc.tile_pool(name="cache", bufs=CACHE_SLOTS))
                wffp = ctx.enter_context(tc.tile_pool(name="wff", bufs=4))
            else:
                p1p = ctx.enter_context(tc.tile_pool(name="p1", bufs=4))
            p2p = ctx.enter_context(tc.tile_pool(name="p2", bufs=3))
            stp = ctx.enter_context(tc.tile_pool(name="stats", bufs=3))
            psS = ctx.enter_context(tc.tile_pool(name="psS", bufs=5, space="PSUM"))
            psC = ctx.enter_context(tc.tile_pool(name="psC", bufs=1, space="PSUM"))

            # ---- constants ----
            ones128 = consts.tile([P, 1], FDT)
            nc.vector.memset(ones128[:], 1.0)
            one11 = consts.tile([1, 1], F32)
            nc.vector.memset(one11[:], 1.0)
            ones_1x32_bf = consts.tile([1, L], BF16)
            nc.vector.memset(ones_1x32_bf[:], 1.0)
            ones_1x128 = consts.tile([1, P], F32)
            nc.vector.memset(ones_1x128[:], 1.0)
            i32sb = consts.tile([L, L], F32)
            nc.sync.dma_start(i32sb[:], i32d[:])
            if feat_bf16:
                i32bsb = consts.tile([L, L], BF16)
                nc.sync.dma_start(i32bsb[:], i32bd[:])
            eps_sb = consts.tile([P, 1], F32)
            nc.vector.memset(eps_sb[:], LN_EPS)

            # ---- weights to SBUF ----
            w_sp1t_sb = wpool.tile([P, DC, H], BF16)
            nc.sync.dma_start(w_sp1t_sb[:], w_sp1t.ap().rearrange("(c p) n -> p c n", p=P))
            w_sp2t_sb = wpool.tile([P, HC, 3], BF16)
            nc.sync.dma_start(w_sp2t_sb[:], w_sp2t.ap().rearrange("(c p) n -> p c n", p=P))
            w_rs1t_sb = wpool.tile([P, 2 * DC, H], BF16)
            nc.sync.dma_start(w_rs1t_sb[:], w_rs1t.ap().rearrange("(c p) n -> p c n", p=P))
            if not feat_bf16:
                w_fft_sb = wpool.tile([P, 2 * DC, D], BF16)
                nc.sync.dma_start(w_fft_sb[:], w_fft_re)
            w_gatet_sb = wpool.tile([P, DC, D], BF16)
            nc.sync.dma_start(w_gatet_sb[:], w_gatet.ap().rearrange("(c p) n -> p c n", p=P))
            b_sp1t_sb = wpool.tile([P, HC], F32)
            nc.sync.dma_start(b_sp1t_sb[:], b_sp1t[:])

            b_sp2_sb = consts.tile([1, 3], F32)
            nc.sync.dma_start(b_sp2_sb[:], b_sp2[:])
            b_rs1_sb = consts.tile([1, H], F32)
            nc.sync.dma_start(b_rs1_sb[:], b_rs1[:])
            w_rs2r_sb = consts.tile([1, H], F32)
            nc.sync.dma_start(w_rs2r_sb[:], w_rs2r[:])
            dlb_sb = consts.tile([1, L], F32)
            nc.sync.dma_start(dlb_sb[:], dlb[:])
            dgb_sb = consts.tile([1, L], F32)
            nc.sync.dma_start(dgb_sb[:], dgb[:])
            b_ff_sb = consts.tile([1, D], F32)
            nc.sync.dma_start(b_ff_sb[:], b_ff[:])
            lnffg_sb = consts.tile([1, D], F32)
            nc.sync.dma_start(lnffg_sb[:], lnffg[:])
            lnffb_sb = consts.tile([1, D], F32)
            nc.sync.dma_start(lnffb_sb[:], lnffb[:])
            b_gate_sb = consts.tile([1, D], F32)
            nc.sync.dma_start(b_gate_sb[:], b_gate[:])

            # w_rs2 broadcast to 32 partitions (one-time)
            pw32 = psS.tile([L, H], F32, tag="ps")
            ones_1x32_f = consts.tile([1, L], F32)
            nc.vector.memset(ones_1x32_f[:], 1.0)
            nc.tensor.matmul(pw32[:], lhsT=ones_1x32_f[:], rhs=w_rs2r_sb[:],
                             start=True, stop=True)
            w_rs2b = consts.tile([L, H], F32)
            nc.scalar.copy(w_rs2b[:], pw32[:])

            if apply_ln_affine:
                lng_sb = consts.tile([1, D], F32)
                nc.sync.dma_start(lng_sb[:], lng[:])
                lnb_sb = consts.tile([1, D], F32)
                nc.sync.dma_start(lnb_sb[:], lnb[:])
                g_bcast = bpool.tile([P, D], F32)
                b_bcast = bpool.tile([P, D], F32)
                for h2 in range(2):
                    pgb = psS.tile([P, 512], F32, tag="ps")
                    nc.tensor.matmul(pgb[:], lhsT=ones_1x128[:],
                                     rhs=lng_sb[:, h2 * 512:(h2 + 1) * 512],
                                     start=True, stop=True)
                    nc.scalar.copy(g_bcast[:, h2 * 512:(h2 + 1) * 512], pgb[:])
                    pbb = psS.tile([P, 512], F32, tag="ps")
                    nc.tensor.matmul(pbb[:], lhsT=ones_1x128[:],
                                     rhs=lnb_sb[:, h2 * 512:(h2 + 1) * 512],
                                     start=True, stop=True)
                    nc.scalar.copy(b_bcast[:, h2 * 512:(h2 + 1) * 512], pbb[:])

            CBD = BF16 if feat_bf16 else F32
            c_bcast = [bpool.tile([P, D], CBD, tag=f"cb{b}", name=f"c_bcast{b}")
                       for b in range(BPC)]
            ctxT_bf = [bpool.tile([P, DC], BF16, tag=f"ctxT{b}", name=f"ctxT{b}")
                       for b in range(BPC)]

            def pe_transpose(out_psum, in_sb, ident):
                if use_pe_transpose:
                    nc.tensor.transpose(out_psum, in_sb, ident)
                else:
                    nc.tensor.matmul(out_psum, lhsT=in_sb, rhs=ident,
                                     start=True, stop=True)

            # identity for transposing FDT-typed tiles
            i32f = i32bsb if feat_bf16 else i32sb

            # ================= PASS 1: context mean =================
            xtiles = [[None] * NT for _ in range(BPC)]

            def pass1(b):
                pc = psC.tile([1, D], F32, tag="pc")  # 2 banks
                for i in range(NT):
                    if feat_bf16:
                        x = cachep.tile([P, D], FDT, tag="xc", name=f"x_{b}_{i}")
                    else:
                        x = p1p.tile([P, D], FDT, tag="x", name=f"x_{b}_{i}")
                    xtiles[b][i] = x
                    nc.sync.dma_start(x[:], feat[i * P:(i + 1) * P, b, :])
                    for h2 in range(2):
                        nc.tensor.matmul(pc[:, h2 * 512:(h2 + 1) * 512],
                                         lhsT=ones128[:],
                                         rhs=x[:, h2 * 512:(h2 + 1) * 512],
                                         start=(i == 0), stop=(i == NT - 1))
                ctx_row = scp.tile([1, D], F32, tag="ctxrow")
                nc.scalar.mul(ctx_row[:], pc[:], 1.0 / S)
                pct = psS.tile([P, DC], F32, tag="ps")
                for j in range(DC):
                    pe_transpose(pct[:, j:j + 1],
                                 ctx_row[:, j * P:(j + 1) * P], one11[:])
                nc.scalar.copy(ctxT_bf[b][:], pct[:])

            # ================= SMALL COMPUTE per b =================
            def small_compute(b):
                # --- span MLP ---
                psp = psS.tile([P, HC], F32, tag="ps")
                for jh in range(HC):
                    for jd in range(DC):
                        nc.tensor.matmul(
                            psp[:, jh:jh + 1],
                            lhsT=w_sp1t_sb[:, jd, jh * P:(jh + 1) * P],
                            rhs=ctxT_bf[b][:, jd:jd + 1],
                            start=(jd == 0), stop=(jd == DC - 1))
                sp_hT = scp.tile([P, HC], BF16, tag="sphT")
                for jh in range(HC):
                    nc.scalar.activation(sp_hT[:, jh:jh + 1], psp[:, jh:jh + 1],
                                         AF.Gelu, bias=b_sp1t_sb[:, jh:jh + 1])
                psl = psS.tile([1, 3], F32, tag="ps")
                for jh in range(HC):
                    nc.tensor.matmul(psl[:], lhsT=sp_hT[:, jh:jh + 1],
                                     rhs=w_sp2t_sb[:, jh, :],
                                     start=(jh == 0), stop=(jh == HC - 1))
                sl = rows.tile([1, 3], F32, tag="sl")
                nc.vector.tensor_add(sl[:], psl[:], b_sp2_sb[:])
                mx = stp.tile([1, 1], F32, tag="mx")
                nc.vector.reduce_max(mx[:], sl[:], AX.X)
                e3 = rows.tile([1, 3], F32, tag="e3")
                nc.vector.tensor_scalar(e3[:], sl[:], mx[:], None, op0=ALU.subtract)
                nc.scalar.activation(e3[:], e3[:], AF.Exp)
                z3 = stp.tile([1, 1], F32, tag="z3")
                nc.vector.reduce_sum(z3[:], e3[:], AX.X)
                rz3 = stp.tile([1, 1], F32, tag="rz3")
                nc.vector.reciprocal(rz3[:], z3[:])
                sw = rows.tile([1, 3], F32, tag="sw")
                nc.vector.tensor_scalar(sw[:], e3[:], rz3[:], None, op0=ALU.mult)

                # --- ctx part of relevance-MLP (shared by both pools) ---
                pcr = psS.tile([1, H], F32, tag="ps")
                for jd in range(DC):
                    nc.tensor.matmul(pcr[:], lhsT=ctxT_bf[b][:, jd:jd + 1],
                                     rhs=w_rs1t_sb[:, DC + jd, :],
                                     start=(jd == 0), stop=(jd == DC - 1))
                ctx_rs_bf = scp.tile([1, H], BF16, tag="ctxrs")
                nc.vector.tensor_add(ctx_rs_bf[:], pcr[:], b_rs1_sb[:])

                # --- two relevance pools ---
                featT = []
                for pi, dbias in enumerate([dlb_sb, dgb_sb]):
                    hist = scp.tile([L, D], FDT, tag="hist")
                    if pi == 0:
                        nc.sync.dma_start(hist[:], feat[S - L:S, b, :])
                    else:
                        nc.sync.dma_start(hist[:], feat[0:S:P, b, :])
                    if feat_bf16:
                        hist_bf = hist
                    else:
                        hist_bf = scp.tile([L, D], BF16, tag="histbf")
                        nc.scalar.copy(hist_bf[:], hist[:])
                    histT_bf = scp.tile([P, DC, L], BF16, tag="histT")
                    for jd in range(DC):
                        pt = psS.tile([P, L], F32, tag="ps")
                        pe_transpose(pt[:], hist[:, jd * P:(jd + 1) * P],
                                     i32f[:])
                        nc.scalar.copy(histT_bf[:, jd, :], pt[:])
                    ph = psS.tile([L, H], F32, tag="ps")
                    for jd in range(DC):
                        nc.tensor.matmul(ph[:], lhsT=histT_bf[:, jd, :],
                                         rhs=w_rs1t_sb[:, jd, :],
                                         start=(jd == 0), stop=False)
                    nc.tensor.matmul(ph[:], lhsT=ones_1x32_bf[:],
                                     rhs=ctx_rs_bf[:], start=False, stop=True)
                    h_sb = scp.tile([L, H], F32, tag="h")
                    nc.scalar.activation(h_sb[:], ph[:], AF.Gelu)
                    hw = scp.tile([L, H], F32, tag="hw")
                    sc_col = stp.tile([L, 1], F32, tag="sccol")
                    if use_ttr:
                        nc.vector.tensor_tensor_reduce(
                            out=hw[:], in0=h_sb[:], in1=w_rs2b[:], scale=1.0,
                            scalar=0.0, op0=ALU.mult, op1=ALU.add,
                            accum_out=sc_col[:])
                    else:
                        nc.vector.tensor_mul(hw[:], h_sb[:], w_rs2b[:])
                        nc.vector.reduce_sum(sc_col[:], hw[:], AX.X)
                    pst = psS.tile([1, L], F32, tag="ps")
                    pe_transpose(pst[:], sc_col[:], i32sb[:])
                    srow = rows.tile([1, L], F32, tag="srow")
                    nc.vector.tensor_add(srow[:], pst[:], dbias[:])
                    mx2 = stp.tile([1, 1], F32, tag="mx2")
                    nc.vector.reduce_max(mx2[:], srow[:], AX.X)
                    e32 = rows.tile([1, L], F32, tag="e32")
                    nc.vector.tensor_scalar(e32[:], srow[:], mx2[:], None,
                                            op0=ALU.subtract)
                    nc.scalar.activation(e32[:], e32[:], AF.Exp)
                    z32 = stp.tile([1, 1], F32, tag="z32")
                    nc.vector.reduce_sum(z32[:], e32[:], AX.X)
                    rz32 = stp.tile([1, 1], F32, tag="rz32")
                    nc.vector.reciprocal(rz32[:], z32[:])
                    wrow = rows.tile([1, L], F32, tag="wrow")
                    nc.vector.tensor_scalar(wrow[:], e32[:], rz32[:], None,
                                            op0=ALU.mult)
                    pwc = psS.tile([L, 1], F32, tag="ps")
                    pe_transpose(pwc[:], wrow[:], one11[:])
                    wcol_bf = stp.tile([L, 1], BF16, tag="wcol")
                    nc.scalar.copy(wcol_bf[:], pwc[:])
                    pft = psS.tile([P, DC], F32, tag="ps")
                    for jd in range(DC):
                        nc.tensor.matmul(pft[:, jd:jd + 1],
                                         lhsT=hist_bf[:, jd * P:(jd + 1) * P],
                                         rhs=wcol_bf[:], start=True, stop=True)
                    fT = scp.tile([P, DC], BF16, tag=f"featT{pi}",
                                  name=f"featT{pi}")
                    nc.scalar.copy(fT[:], pft[:])
                    featT.append(fT)

                # --- fusion ff ---
                pAB = []
                for pi in range(2):
                    pAh = [psS.tile([1, 512], F32, tag="ps", name=f"pA{pi}{h2}")
                           for h2 in range(2)]
                    for jd in range(DC):
                        if feat_bf16:
                            wff_c = wffp.tile([P, D], BF16, tag="wff",
                                              name=f"wffc{pi}{jd}")
                            nc.sync.dma_start(wff_c[:], w_fft_re[:, pi * DC + jd, :])
                        else:
                            wff_c = w_fft_sb[:, pi * DC + jd, :]
                        for h2 in range(2):
                            nc.tensor.matmul(
                                pAh[h2][:], lhsT=featT[pi][:, jd:jd + 1],
                                rhs=wff_c[:, h2 * 512:(h2 + 1) * 512],
                                start=(jd == 0), stop=(jd == DC - 1))
                    pAB.append(pAh)
                t1 = scp.tile([1, D], F32, tag="rowA")
                t2 = scp.tile([1, D], F32, tag="rowB")
                for h2 in range(2):
                    nc.vector.tensor_scalar(t1[:, h2 * 512:(h2 + 1) * 512],
                                            pAB[0][h2][:], sw[:, 1:2], None,
                                            op0=ALU.mult)
                    nc.vector.tensor_scalar(t2[:, h2 * 512:(h2 + 1) * 512],
                                            pAB[1][h2][:], sw[:, 2:3], None,
                                            op0=ALU.mult)
                nc.vector.tensor_add(t1[:], t1[:], t2[:])
                nc.vector.tensor_add(t1[:], t1[:], b_ff_sb[:])
                ff = scp.tile([1, D], F32, tag="rowC")
                nc.scalar.activation(ff[:], t1[:], AF.Gelu)
                # LN over free axis
                smu = stp.tile([1, 1], F32, tag="smu")
                nc.vector.reduce_sum(smu[:], ff[:], AX.X)
                mu = stp.tile([1, 1], F32, tag="mu")
                nc.vector.tensor_scalar(mu[:], smu[:], 1.0 / D, None, op0=ALU.mult)
                ffc = scp.tile([1, D], F32, tag="rowB")
                nc.vector.tensor_scalar(ffc[:], ff[:], mu[:], None, op0=ALU.subtract)
                ffsq = scp.tile([1, D], F32, tag="rowA")
                vs = stp.tile([1, 1], F32, tag="vs")
                if use_ttr:
                    nc.vector.tensor_tensor_reduce(
                        out=ffsq[:], in0=ffc[:], in1=ffc[:], scale=1.0,
                        scalar=0.0, op0=ALU.mult, op1=ALU.add, accum_out=vs[:])
                else:
                    nc.vector.tensor_mul(ffsq[:], ffc[:], ffc[:])
                    nc.vector.reduce_sum(vs[:], ffsq[:], AX.X)
                stdv = stp.tile([1, 1], F32, tag="stdv")
                nc.scalar.activation(stdv[:], vs[:], AF.Sqrt, bias=eps_sb[0:1, :],
                                     scale=1.0 / D)
                rstd = stp.tile([1, 1], F32, tag="rstd")
                nc.vector.reciprocal(rstd[:], stdv[:])
                fused = scp.tile([1, D], F32, tag="rowD")
                nc.vector.tensor_scalar(fused[:], ffc[:], rstd[:], None, op0=ALU.mult)
                nc.vector.tensor_mul(fused[:], fused[:], lnffg_sb[:])
                nc.vector.tensor_add(fused[:], fused[:], lnffb_sb[:])

                # --- gate ---
                pfT = psS.tile([P, DC], F32, tag="ps")
                for jd in range(DC):
                    pe_transpose(pfT[:, jd:jd + 1],
                                 fused[:, jd * P:(jd + 1) * P], one11[:])
                fusedT_bf = scp.tile([P, DC], BF16, tag="fusedT")
                nc.scalar.copy(fusedT_bf[:], pfT[:])
                gpre = scp.tile([1, D], F32, tag="rowA")
                for h2 in range(2):
                    pg = psS.tile([1, 512], F32, tag="ps")
                    for jd in range(DC):
                        nc.tensor.matmul(pg[:],
                                         lhsT=fusedT_bf[:, jd:jd + 1],
                                         rhs=w_gatet_sb[:, jd, h2 * 512:(h2 + 1) * 512],
                                         start=(jd == 0), stop=(jd == DC - 1))
                    nc.vector.tensor_add(gpre[:, h2 * 512:(h2 + 1) * 512], pg[:],
                                         b_gate_sb[:, h2 * 512:(h2 + 1) * 512])
                gate = scp.tile([1, D], F32, tag="rowB")
                nc.scalar.activation(gate[:], gpre[:], AF.Sigmoid)
                c_row = scp.tile([1, D], F32, tag="rowA")
                nc.vector.tensor_mul(c_row[:], fused[:], gate[:])
                for h2 in range(2):
                    pcb = psS.tile([P, 512], F32, tag="ps")
                    nc.tensor.matmul(pcb[:], lhsT=ones_1x128[:],
                                     rhs=c_row[:, h2 * 512:(h2 + 1) * 512],
                                     start=True, stop=True)
                    nc.scalar.copy(c_bcast[b][:, h2 * 512:(h2 + 1) * 512], pcb[:])

            # ================= PASS 2: LN(x + c) =================
            def pass2(b):
                for i in range(NT):
                    if feat_bf16:
                        x = xtiles[b][i]
                    else:
                        x = p2p.tile([P, D], F32, tag="x2", name=f"x2_{b}_{i}")
                        nc.sync.dma_start(x[:], feat[i * P:(i + 1) * P, b, :])
                    xa = p2p.tile([P, D], F32, tag="xa")
                    o = p2p.tile([P, D], F32, tag="o")
                    if use_ttr and use_sq_accum:
                        st = stp.tile([P, 8], F32, tag="st")
                        nc.vector.tensor_tensor_reduce(
                            out=xa[:], in0=x[:], in1=c_bcast[b][:], scale=1.0,
                            scalar=0.0, op0=ALU.add, op1=ALU.add,
                            accum_out=st[:, 0:1])
                        nc.scalar.activation(o[:], xa[:], AF.Square,
                                             accum_out=st[:, 1:2])
                        # mean, E[x^2], var, rstd
                        nc.vector.tensor_scalar(st[:, 2:3], st[:, 0:1], 1.0 / D,
                                                None, op0=ALU.mult)
                        nc.vector.tensor_scalar(st[:, 3:4], st[:, 1:2], 1.0 / D,
                                                None, op0=ALU.mult)
                        nc.vector.tensor_mul(st[:, 4:5], st[:, 2:3], st[:, 2:3])
                        nc.vector.tensor_sub(st[:, 5:6], st[:, 3:4], st[:, 4:5])
                        nc.scalar.activation(st[:, 6:7], st[:, 5:6], AF.Sqrt,
                                             bias=eps_sb[:])
                        rst = stp.tile([P, 1], F32, tag="rst")
                        nc.vector.reciprocal(rst[:], st[:, 6:7])
                        nc.vector.tensor_scalar(o[:], xa[:], st[:, 2:3],
                                                rst[:], op0=ALU.subtract,
                                                op1=ALU.mult)
                    else:
                        nc.vector.tensor_add(xa[:], x[:], c_bcast[b][:])
                        stats = stp.tile([P, 2, 6], F32, tag="bst")
                        xa3 = xa.rearrange("p (s f) -> p s f", f=512)
                        for si in range(2):
                            nc.vector.bn_stats(stats[:, si, :], xa3[:, si, :])
                        mv = stp.tile([P, 2], F32, tag="mv")
                        nc.vector.bn_aggr(mv[:], stats[:])
                        rst = stp.tile([P, 1], F32, tag="rst2")
                        nc.scalar.activation(rst[:], mv[:, 1:2], AF.Sqrt,
                                             bias=eps_sb[:])
                        nc.vector.reciprocal(rst[:], rst[:])
                        nc.vector.tensor_scalar(o[:], xa[:], mv[:, 0:1],
                                                rst[:], op0=ALU.subtract,
                                                op1=ALU.mult)
                    if apply_ln_affine:
                        nc.vector.tensor_mul(o[:], o[:], g_bcast[:])
                        nc.vector.tensor_add(o[:], o[:], b_bcast[:])
                    # Output DMAs go on the gpsimd queue: the sync queue
                    # carries b1's input loads, which block on cache-slot
                    # release by this pass — same-queue ordering would
                    # deadlock.
                    if feat_bf16:
                        nc.gpsimd.dma_start(out[i * P:(i + 1) * P, b, :], o[:])
                    else:
                        nc.sync.dma_start(out[i * P:(i + 1) * P, b, :], o[:])

            def whole_kernel():
                for b in range(BPC):
                    pass1(b)
                    small_compute(b)
                    pass2(b)

            if reps > 1:
                with tc.For_i(0, reps, 1):
                    whole_kernel()
            else:
                whole_kernel()

            if os.environ.get("KERNEL_DEBUG_SIZES"):
                pools = [consts, wpool, bpool, scp, rows, p2p, stp]
                if feat_bf16:
                    pools += [cachep, wffp]
                else:
                    pools += [p1p]
                tot = 0
                for pl in pools:
                    sz = pl.current_size() / 128 / 1024
                    tot += sz
                    print(f"pool {pl.name}: {sz:.1f} KB/partition")
                print(f"TOTAL SBUF: {tot:.1f} KB/partition of "
                      f"{nc.SBUF_PARTITION_SIZE_BYTES/1024:.0f}")
                for pl in [psS, psC]:
                    print(f"pool {pl.name}: {pl.current_size()/128/2048:.1f} banks")

    nc.compile()
    return nc


_CACHE = {}


def _get_nc(apply_ln_affine: bool):
    key = (apply_ln_affine, FLAGS["feat_bf16"], FLAGS["use_ttr"],
           FLAGS["use_sq_accum"], FLAGS["use_pe_transpose"])
    if key not in _CACHE:
        _CACHE[key] = _build(apply_ln_affine, *key[1:])
    return _CACHE[key]


def build_in_maps(inputs):
    features = np.asarray(inputs["features"], np.float32)
    f32 = lambda a: np.ascontiguousarray(np.asarray(a, np.float32))
    bf = lambda a: np.ascontiguousarray(
        np.asarray(a, np.float32).astype(ml_dtypes.bfloat16))

    b_rs2 = f32(inputs["b_rs2"])
    ln_g, ln_b = f32(inputs["ln_g"]), f32(inputs["ln_b"])
    dl = float(np.asarray(inputs["decay_local"]))
    dg = float(np.asarray(inputs["decay_global"]))
    apply_ln_affine = not (np.all(ln_g == 1.0) and np.all(ln_b == 0.0))

    pos = np.arange(L, dtype=np.float64)
    dlb = (np.log(dl ** (L - 1 - pos) + 1e-8) + float(b_rs2[0])).astype(np.float32)
    dgb = (np.log(dg ** (L - 1 - pos) + 1e-8) + float(b_rs2[0])).astype(np.float32)

    common = {
        "w_sp1t": bf(f32(inputs["w_sp1"]).T),
        "b_sp1t": f32(f32(inputs["b_sp1"]).reshape(HC, P).T),
        "w_sp2t": bf(f32(inputs["w_sp2"]).T),
        "b_sp2": f32(inputs["b_sp2"]).reshape(1, 3),
        "w_rs1t": bf(f32(inputs["w_rs1"]).T),
        "b_rs1": f32(inputs["b_rs1"]).reshape(1, H),
        "w_rs2r": f32(inputs["w_rs2"]).reshape(1, H),
        "dlb": dlb.reshape(1, L),
        "dgb": dgb.reshape(1, L),
        "w_fft": bf(f32(inputs["w_ff"]).T),
        "b_ff": f32(inputs["b_ff"]).reshape(1, D),
        "lnffg": f32(inputs["ln_ff_g"]).reshape(1, D),
        "lnffb": f32(inputs["ln_ff_b"]).reshape(1, D),
        "w_gatet": bf(f32(inputs["w_gate"]).T),
        "b_gate": f32(inputs["b_gate"]).reshape(1, D),
        "i32": np.eye(L, dtype=np.float32),
    }
    if FLAGS["feat_bf16"]:
        common["i32b"] = np.eye(L, dtype=np.float32).astype(ml_dtypes.bfloat16)
    if apply_ln_affine:
        common["lng"] = ln_g.reshape(1, D)
        common["lnb"] = ln_b.reshape(1, D)

    if FLAGS["feat_bf16"]:
        features_dev = features.astype(ml_dtypes.bfloat16)
    else:
        features_dev = features

    in_maps = []
    for c in range(NC):
        m = dict(common)
        m["features"] = np.ascontiguousarray(
            features_dev[:, c * BPC:(c + 1) * BPC, :])
        in_maps.append(m)
    return in_maps


def kernel(features, w_sp1, b_sp1, w_sp2, b_sp2, w_rs1, b_rs1, w_rs2, b_rs2,
           decay_local, decay_global, w_ff, b_ff, ln_ff_g, ln_ff_b,
           w_gate, b_gate, ln_g, ln_b, _trace=False):
    inputs = dict(features=features, w_sp1=w_sp1, b_sp1=b_sp1, w_sp2=w_sp2,
                  b_sp2=b_sp2, w_rs1=w_rs1, b_rs1=b_rs1, w_rs2=w_rs2,
                  b_rs2=b_rs2, decay_local=decay_local,
                  decay_global=decay_global, w_ff=w_ff, b_ff=b_ff,
                  ln_ff_g=ln_ff_g, ln_ff_b=ln_ff_b, w_gate=w_gate,
                  b_gate=b_gate, ln_g=ln_g, ln_b=ln_b)
    features = np.asarray(features, np.float32)
    ln_g_np = np.asarray(ln_g, np.float32)
    ln_b_np = np.asarray(ln_b, np.float32)
    apply_ln_affine = not (np.all(ln_g_np == 1.0) and np.all(ln_b_np == 0.0))
    nc = _get_nc(apply_ln_affine)
    in_maps = build_in_maps(inputs)

    res = run_bass_kernel_spmd(nc, in_maps, core_ids=list(range(NC)),
                               trace=_trace)
    output = np.concatenate([res.results[c]["out"] for c in range(NC)], axis=1)
    attention_weights = np.full((S, B), 1.0 / S, dtype=features.dtype)
    if _trace:
        kernel.last_exec_time_ns = res.exec_time_ns
        kernel.last_trace = res.instructions_and_trace
    return output, attention_weights
